# revision 48
# baseline (speedup 1.0000x reference)
"""AdaLN attention block (DiT-style) on 8 TRN2 NeuronCores.

Sharding: 8 cores = 4 batches x 2 token-halves, no collectives. Core c handles
batch c//2 and query-token half c%2: layernorm1 and k/v are computed over the
full (permuted) sequence, everything else only for the own 512 query rows.

Device layout is feature-major (activations transposed, [d, n]). X @ W runs
with W column-tiles stationary and X^T moving, producing Y^T directly.
LayerNorm statistics use ones-vector matmuls (partition-axis sums on the PE);
the AdaLN modulate is h = x*A + B with rank-1 A/B built by K=1 outer-product
matmuls into PSUM. Softmax skips max-subtraction (fp32 exp is safe for this
distribution); the denominator is a ones-column appended to the attn@v
stationary operand; normalization is folded in per head via a broadcast
reciprocal.
"""

import numpy as np
from contextlib import ExitStack

import concourse.bass as bass
import concourse.bacc as bacc
import concourse.mybir as mybir
from concourse import tile
from concourse.bass_utils import run_bass_kernel_spmd

P = 128
D = 1024
N = 1024
NQ = 512
H = 16
DH = 64
MLPD = 4096
EPS = 1e-6
NCORES = 8

F32 = mybir.dt.float32
BF16 = mybir.dt.bfloat16
AF = mybir.ActivationFunctionType
ALU = mybir.AluOpType

KT = D // P           # 8 contraction tiles over D
MT = MLPD // P        # 32 tiles over MLP dim


def _r(ap):
    return ap


def build():
    nc = bacc.Bacc("TRN2", target_bir_lowering=False, debug=False,
                   num_devices=NCORES)

    xT = nc.dram_tensor("xT", [D, N], BF16, kind="ExternalInput")
    crow = nc.dram_tensor("crow", [1, D], F32, kind="ExternalInput")
    Wq = nc.dram_tensor("Wq", [D, D], BF16, kind="ExternalInput")
    Wkv = nc.dram_tensor("Wkv", [D, 2 * D], BF16, kind="ExternalInput")
    Wo = nc.dram_tensor("Wo", [D, D], BF16, kind="ExternalInput")
    W1 = nc.dram_tensor("W1", [D, MLPD], BF16, kind="ExternalInput")
    W2 = nc.dram_tensor("W2", [MLPD, D], BF16, kind="ExternalInput")
    Wada = nc.dram_tensor("Wada", [D, 6 * D], BF16, kind="ExternalInput")
    bada_r = nc.dram_tensor("bada_r", [1, 6 * D], F32, kind="ExternalInput")
    bq_c = nc.dram_tensor("bq_c", [P, KT], F32, kind="ExternalInput")
    bk_c = nc.dram_tensor("bk_c", [P, KT], F32, kind="ExternalInput")
    bv_c = nc.dram_tensor("bv_c", [P, KT], BF16, kind="ExternalInput")
    bo_r = nc.dram_tensor("bo_r", [1, D], F32, kind="ExternalInput")
    b1_c = nc.dram_tensor("b1_c", [P, MT], F32, kind="ExternalInput")
    b2_c = nc.dram_tensor("b2_c", [P, KT], F32, kind="ExternalInput")
    yT = nc.dram_tensor("yT", [D, NQ], BF16, kind="ExternalOutput")

    with tile.TileContext(nc) as tc, ExitStack() as root:
        const = root.enter_context(tc.tile_pool(name="const", bufs=1))
        rootrows = root.enter_context(tc.tile_pool(name="rootrows", bufs=1))
        mod_row = rootrows.tile([1, 6 * D], BF16, name='mod_row')
        bad_full = rootrows.tile([1, 6 * D], F32, name='bad_full')

        ones_col = const.tile([P, 1], BF16, name='ones_col')
        nc.vector.memset(ones_col[:], 1.0)
        ones_row = const.tile([1, NQ], BF16, name='ones_row')
        nc.vector.memset(ones_row[:], 1.0)
        dall_row = const.tile([1, P], BF16, name='dall_row')
        nc.vector.memset(dall_row[:], float(D))
        epsD = const.tile([1, 1], F32, name='epsD')
        nc.vector.memset(epsD[:], EPS * D * D)

        # bias columns are loaded on the gpsimd queue AFTER the x tiles so
        # the latency-critical transfers go first
        bqT = const.tile([P, KT], F32, name='bqT')
        bkT = const.tile([P, KT], F32, name='bkT')
        bvT = const.tile([P, KT], BF16, name='bvT')
        b1T = const.tile([P, MT], F32, name='b1T')
        b2T = const.tile([P, KT], F32, name='b2T')
        bo_row = const.tile([1, D], F32, name='bo_row')
        bqT_s = const.tile([P, KT], F32, name='bqT_s')

        def load_consts():
            nc.gpsimd.dma_start(bqT[:], bq_c[:])
            nc.gpsimd.dma_start(bkT[:], bk_c[:])
            nc.gpsimd.dma_start(bvT[:], bv_c[:])
            nc.gpsimd.dma_start(b1T[:], b1_c[:])
            nc.gpsimd.dma_start(b2T[:], b2_c[:])
            nc.gpsimd.dma_start(bo_row[:], bo_r[:])
            nc.vector.tensor_scalar_mul(bqT_s[:], bqT[:], DH ** -0.5)

        def cols_from_row(row_ap, dst, psum_pool, plus1=False):
            """[1, n*128] row -> [128, n] column tile via K=1 matmuls."""
            n = dst.shape[-1]
            ps = psum_pool.tile([P, n], F32, tag="colps", name='colps')
            for j in range(n):
                nc.tensor.matmul(ps[:, j:j + 1],
                                 lhsT=_r(row_ap[0:1, j * P:(j + 1) * P]),
                                 rhs=_r(ones_row[0:1, 0:1]),
                                 start=True, stop=True)
            if plus1:
                nc.vector.tensor_scalar_add(dst[:], ps[:], 1.0)
            else:
                nc.vector.tensor_copy(dst[:], ps[:])
            return dst

        csT = const.tile([P, KT], BF16, name='csT')
        gmsaT = const.tile([P, KT], F32, name='gmsaT')
        gmlpT = const.tile([P, KT], F32, name='gmlpT')
        S1T = const.tile([P, KT], F32, name='S1T')
        sh1T = const.tile([P, KT], F32, name='sh1T')
        S2T = const.tile([P, KT], F32, name='S2T')
        sh2T = const.tile([P, KT], F32, name='sh2T')

        # persistent activation arrays (distinct tag per tile, 1 buf each)
        op_cm = tc.tile_pool(name="op", bufs=1, side='left')
        op_ = op_cm.__enter__()
        outT = [op_.tile([P, NQ], BF16, tag=f"o{k}", name=f"o{k}")
                for k in range(KT)]
        wo_cm = tc.tile_pool(name="wop", bufs=1, side='left')
        wop = wo_cm.__enter__()
        wot = [[wop.tile([P, NQ], BF16, tag=f"wo{g}_{k}", name=f"wo{g}_{k}")
                for k in range(KT)] for g in range(2)]
        hT_cm = tc.tile_pool(name="hTp", bufs=1, side='left')
        hTp = hT_cm.__enter__()
        hTa = [hTp.tile([P, NQ], BF16, tag=f"ha{k}", name=f"ha{k}")
               for k in range(KT)]
        hTb = [hTp.tile([P, NQ], BF16, tag=f"hb{k}", name=f"hb{k}")
               for k in range(KT)]

        # ---------------- phase 0+1: mod vector & ln1 ----------------
        with ExitStack() as ph:
            rows = ph.enter_context(tc.tile_pool(name="p0rows", bufs=1))
            xpool = ph.enter_context(tc.tile_pool(name="p0x", bufs=1))
            sqpool = ph.enter_context(tc.tile_pool(name="p0sq", bufs=4))
            wpool = ph.enter_context(tc.tile_pool(name="p0w", bufs=8))

            # c + bada first, on the scalar HWDGE queue so they are not
            # stuck behind the bulk x/Wada transfers
            c_sb = rows.tile([1, D], F32, name='c_sb')
            nc.scalar.dma_start(c_sb[:], crow[:])
            nc.scalar.dma_start(bad_full[:], bada_r[:])

            xt = [xpool.tile([P, N], BF16, tag=f"x{k}", name=f"x{k}")
                  for k in range(KT)]
            for k in range(KT):
                eng = nc.sync if k % 2 == 0 else nc.gpsimd
                eng.dma_start(xt[k][:], xT[k * P:(k + 1) * P, :])
            load_consts()

            with ExitStack() as sec:
                pscol = sec.enter_context(
                    tc.tile_pool(name="pscol", bufs=1, space="PSUM"))
                psmod = sec.enter_context(
                    tc.tile_pool(name="psmod", bufs=3, space="PSUM"))
                psstat = sec.enter_context(
                    tc.tile_pool(name="psstat", bufs=2, space="PSUM"))

                # silu(c) and its column layout
                cs_row = rows.tile([1, D], BF16, name='cs_row')
                nc.scalar.activation(cs_row[:], c_sb[:], AF.Silu)
                cols_from_row(cs_row, csT, pscol)

                # ln1 stats: per 512-chunk, sum and sumsq over d.
                # sum chains land on col-strip 1 (partition 32), sumsq on
                # strip 2 (partition 64): the PE runs them concurrently.
                s_row = rows.tile([1, N], F32, name='s_row')
                t_row = rows.tile([1, N], F32, name='t_row')
                for ch in range(2):
                    sl = slice(ch * NQ, (ch + 1) * NQ)
                    ss = psstat.tile([P, NQ], F32, tag="st_s", name='st_s')
                    sq_ps = psstat.tile([P, NQ], F32, tag="st_q",
                                        name='st_q')
                    for k in range(KT):
                        sq = sqpool.tile([P, NQ], BF16, tag="xsq",
                                         name='xsq')
                        nc.scalar.square(sq[:], xt[k][:, sl])
                        nc.tensor.matmul(ss[32:33, :],
                                         lhsT=_r(ones_col[:]),
                                         rhs=_r(xt[k][:, sl]),
                                         start=(k == 0), stop=(k == KT - 1))
                        nc.tensor.matmul(sq_ps[64:65, :],
                                         lhsT=_r(ones_col[:]),
                                         rhs=_r(sq[:]),
                                         start=(k == 0), stop=(k == KT - 1))
                    nc.vector.tensor_copy(s_row[0:1, sl], ss[32:33, :])
                    # t = D*sumsq - sum^2  (var*D^2, istd = D/sqrt(t+eps*D^2))
                    s2 = rows.tile([1, NQ], F32, name=f's2_{ch}')
                    nc.vector.tensor_mul(s2[:], s_row[0:1, sl],
                                         s_row[0:1, sl])
                    nc.vector.scalar_tensor_tensor(
                        t_row[0:1, sl], sq_ps[64:65, :], float(D), s2[:],
                        ALU.mult, ALU.subtract)

                # mod = silu(c) @ Wada + bada — groups 0-3 (sh/sc_msa) now;
                # groups 4-11 are deferred into phase 2 so their Wada DMA
                # doesn't block the PE stream here. Groups rotate over
                # col-strips 0/1/2 for 3x concurrency.
                wch0 = [wpool.tile([P, 4 * NQ], BF16, tag="wada",
                                   name='wada') for _ in range(KT)]
                for k in range(KT):
                    eng = (nc.sync, nc.gpsimd, nc.scalar)[k % 3]
                    eng.dma_start(wch0[k][:],
                                  Wada[k * P:(k + 1) * P, 0:2 * D])
                for g in range(4):
                    sp = 32 * (g % 3)
                    mp = psmod.tile([P, NQ], F32, tag="modps",
                                    name='modps')
                    for k in range(KT):
                        nc.tensor.matmul(
                            mp[sp:sp + 1, :],
                            lhsT=_r(csT[:, k:k + 1]),
                            rhs=_r(wch0[k][:, g * NQ:(g + 1) * NQ]),
                            start=(k == 0), stop=(k == KT - 1))
                    nc.vector.tensor_add(
                        mod_row[0:1, g * NQ:(g + 1) * NQ],
                        mp[sp:sp + 1, :],
                        bad_full[0:1, g * NQ:(g + 1) * NQ])

                # r = 1/sqrt(t + eps*D^2); istd = D*r (D folded into the
                # broadcast lhsT); b = -mu*istd = -sum*r
                a_row = rows.tile([1, N], BF16, name='a_row')
                nc.scalar.activation(a_row[:], t_row[:],
                                     AF.Abs_reciprocal_sqrt, bias=epsD[:])
                b_row = rows.tile([1, N], BF16, name='b_row')
                nc.vector.scalar_tensor_tensor(
                    b_row[:], s_row[:], -1.0, a_row[:],
                    ALU.mult, ALU.mult)

                # modulation columns for the attention branch
                cols_from_row(mod_row[0:1, 0:D], sh1T, pscol)
                cols_from_row(mod_row[0:1, D:2 * D], S1T, pscol,
                              plus1=True)

            # h = (x*S1*a_bcast) + (b_bcast*S1 + sh1): two fused DVE ops
            # per chunk against shared rank-1 broadcast tiles.
            with ExitStack() as sec:
                psbr = sec.enter_context(
                    tc.tile_pool(name="psbr", bufs=1, space="PSUM"))
                brpool = sec.enter_context(tc.tile_pool(name="brp",
                                                        bufs=1))
                ba = {}
                bb = {}
                for ch in range(2):
                    sl = slice(ch * NQ, (ch + 1) * NQ)
                    bap = psbr.tile([P, NQ], F32, tag=f"ba{ch}",
                                    name=f"ba{ch}")
                    nc.tensor.matmul(bap[:], lhsT=_r(dall_row[:]),
                                     rhs=_r(a_row[0:1, sl]),
                                     start=True, stop=True)
                    ba[ch] = brpool.tile([P, NQ], BF16, tag=f"bas{ch}",
                                         name=f"bas{ch}")
                    nc.vector.tensor_copy(ba[ch][:], bap[:])
                    bbp = psbr.tile([P, NQ], F32, tag=f"bb{ch}",
                                    name=f"bb{ch}")
                    nc.tensor.matmul(bbp[:], lhsT=_r(ones_row[0:1, 0:P]),
                                     rhs=_r(b_row[0:1, sl]),
                                     start=True, stop=True)
                    bb[ch] = brpool.tile([P, NQ], BF16, tag=f"bbs{ch}",
                                         name=f"bbs{ch}")
                    nc.vector.tensor_copy(bb[ch][:], bbp[:])
                for ch in range(2):
                    for k in range(KT):
                        sl = slice(ch * NQ, (ch + 1) * NQ)
                        t1 = sqpool.tile([P, NQ], BF16, tag="t1",
                                         name='t1')
                        nc.vector.scalar_tensor_tensor(
                            t1[:], xt[k][:, sl], S1T[:, k:k + 1],
                            ba[ch][:], ALU.mult, ALU.mult)
                        hdst = (hTa if ch == 0 else hTb)[k]
                        nc.vector.affine_then_add(
                            hdst[:], bb[ch][:], t1[:],
                            scale=S1T[:, k:k + 1], bias=sh1T[:, k:k + 1])

        # ---------------- phase 2: q, k, v projections ----------------
        qkv_cm = tc.tile_pool(name="qkvp", bufs=1, side='right')
        qkvp = qkv_cm.__enter__()
        qTt = [qkvp.tile([P, NQ], BF16, tag=f"q{k}", name=f"q{k}")
               for k in range(KT)]
        kTt = [qkvp.tile([P, N], BF16, tag=f"k{k}", name=f"k{k}")
               for k in range(KT)]
        vRt = [qkvp.tile([P, H * (DH + 1)], BF16, tag=f"v{k}", name=f"v{k}")
               for k in range(KT)]

        wkv_cm = tc.tile_pool(name="wkvp", bufs=1, side='right')
        wkvp = wkv_cm.__enter__()
        wkc = {}   # (g) -> k-part chunks; ('v', vg) -> v-part chunks
        for g in range(2):
            wkc[g] = [wkvp.tile([P, NQ], BF16, tag=f"kg{g}_{k}",
                                name=f"kg{g}_{k}") for k in range(KT)]
            for k in range(KT):
                eng = (nc.sync, nc.gpsimd, nc.scalar)[k % 3]
                eng.dma_start(wkc[g][k][:],
                              Wkv[k * P:(k + 1) * P, g * NQ:(g + 1) * NQ])
        for vg in range(2):
            wkc['v', vg] = [wkvp.tile([P, NQ], BF16, tag=f"vg{vg}_{k}",
                                      name=f"vg{vg}_{k}")
                            for k in range(KT)]
            for k in range(KT):
                eng = (nc.sync, nc.gpsimd, nc.scalar)[k % 3]
                eng.dma_start(wkc['v', vg][k][:],
                              Wkv[k * P:(k + 1) * P,
                                  D + vg * NQ:D + (vg + 1) * NQ])

        prj_cm = tc.tile_pool(name="prjps", bufs=1, space="PSUM",
                              side='right')
        prjps = prj_cm.__enter__()

        def emit_kT(t, ch):
            g, dot = t // 4, t % 4
            sl = slice(ch * NQ, (ch + 1) * NQ)
            p = prjps.tile([P, NQ], F32, tag="prj", name='prj')
            for k in range(KT):
                nc.tensor.matmul(
                    p[:], lhsT=_r(wkc[g][k][:, dot * P:(dot + 1) * P]),
                    rhs=_r((hTa if ch == 0 else hTb)[k][:]),
                    start=(k == 0), stop=(k == KT - 1))
            nc.vector.tensor_scalar_add(kTt[t][:, sl], p[:],
                                        bkT[:, t:t + 1])

        def emit_v(vg, nt):
            p = prjps.tile([P, NQ], F32, tag="prj", name='prj')
            for k in range(KT):
                nc.tensor.matmul(
                    p[:], lhsT=_r(hTa[k][:, nt * P:(nt + 1) * P]
                                  if nt < 4 else
                                  hTb[k][:, (nt - 4) * P:(nt - 3) * P]),
                    rhs=_r(wkc['v', vg][k][:]),
                    start=(k == 0), stop=(k == KT - 1))
            vv = vRt[nt].rearrange("p (h w) -> p h w", w=DH + 1)
            pv = p.rearrange("p (h w) -> p h w", w=DH)
            nc.vector.tensor_copy(vv[:, vg * 8:(vg + 1) * 8, 0:DH], pv[:])

        with ExitStack() as ph:
            wpool = ph.enter_context(tc.tile_pool(name="p2w", bufs=26))
            wadap = ph.enter_context(tc.tile_pool(name="p2wada", bufs=8))
            ps = ph.enter_context(
                tc.tile_pool(name="p2ps", bufs=3, space="PSUM"))
            psmod2 = ph.enter_context(
                tc.tile_pool(name="psmod2", bufs=3, space="PSUM"))
            pscolb = ph.enter_context(
                tc.tile_pool(name="pscolb", bufs=1, space="PSUM"))

            for nt in range(KT):
                vv = vRt[nt].rearrange("p (h w) -> p h w", w=DH + 1)
                nc.vector.memset(vv[:, :, DH:DH + 1], 1.0)

            def stationary_group(wdram, col0, movs, evict, tagp):
                wch = [wpool.tile([P, NQ], BF16, tag=tagp, name=tagp)
                       for _ in range(KT)]
                for k in range(KT):
                    eng = (nc.sync, nc.gpsimd, nc.scalar)[k % 3]
                    eng.dma_start(
                        wch[k][:], wdram[k * P:(k + 1) * P, col0:col0 + NQ])
                for dot in range(4):
                    p = ps.tile([P, NQ], F32, tag="prj", name='prj')
                    for k in range(KT):
                        nc.tensor.matmul(
                            p[:], lhsT=_r(wch[k][:, dot * P:(dot + 1) * P]),
                            rhs=movs[k], start=(k == 0), stop=(k == KT - 1))
                    evict(dot, p)

            # q^T (own rows), scaled by 1/sqrt(DH)
            for g in range(2):
                def ev_q(dot, p, g=g):
                    t = 4 * g + dot
                    nc.vector.tensor_scalar(qTt[t][:], p[:], DH ** -0.5,
                                            bqT_s[:, t:t + 1],
                                            ALU.mult, ALU.add)
                stationary_group(Wq, g * NQ,
                                 [_r(hTa[k][:]) for k in range(KT)],
                                 ev_q, "wst")

            # k^T tiles 0-1 and v-group 0 now; the rest is emitted inside
            # the attention loop as just-in-time full-array work that keeps
            # the PE clock warm
            for t in range(2):
                emit_kT(t, 0)
                emit_kT(t, 1)
            for nt in range(KT):
                emit_v(0, nt)

            # deferred mod groups 4-11 (msa gate + mlp modulation): their
            # Wada traffic and PE work hide behind the projection phase
            for g2 in range(2, 6):
                wch = [wadap.tile([P, 2 * NQ], BF16, tag="wada2",
                                  name='wada2') for _ in range(KT)]
                for k in range(KT):
                    eng = (nc.sync, nc.gpsimd, nc.scalar)[k % 3]
                    eng.dma_start(
                        wch[k][:], Wada[k * P:(k + 1) * P,
                                        g2 * D:(g2 + 1) * D])
                for sub in range(2):
                    g = 2 * g2 + sub
                    sp = 32 * (g % 3)
                    mp = psmod2.tile([P, NQ], F32, tag="modps2",
                                     name='modps2')
                    for k in range(KT):
                        nc.tensor.matmul(
                            mp[sp:sp + 1, :],
                            lhsT=_r(csT[:, k:k + 1]),
                            rhs=_r(wch[k][:, sub * NQ:(sub + 1) * NQ]),
                            start=(k == 0), stop=(k == KT - 1))
                    nc.vector.tensor_add(
                        mod_row[0:1, g * NQ:(g + 1) * NQ],
                        mp[sp:sp + 1, :],
                        bad_full[0:1, g * NQ:(g + 1) * NQ])
            cols_from_row(mod_row[0:1, 2 * D:3 * D], gmsaT, pscolb)
            cols_from_row(mod_row[0:1, 3 * D:4 * D], sh2T, pscolb)
            cols_from_row(mod_row[0:1, 4 * D:5 * D], S2T, pscolb,
                          plus1=True)
            cols_from_row(mod_row[0:1, 5 * D:6 * D], gmlpT, pscolb)


        # ---------------- phase 3: attention ----------------

        def prefetch_wo():
            for g in range(2):
                for k in range(KT):
                    eng = nc.sync if k % 2 == 0 else nc.gpsimd
                    eng.dma_start(
                        wot[g][k][:],
                        Wo[k * P:(k + 1) * P, g * NQ:(g + 1) * NQ])

        with ExitStack() as ph:
            epool = ph.enter_context(tc.tile_pool(name="p3e", bufs=10))
            spool = ph.enter_context(tc.tile_pool(name="p3s", bufs=3))
            ps_sim = ph.enter_context(
                tc.tile_pool(name="ps_sim", bufs=2, space="PSUM"))
            ps_bc = ph.enter_context(
                tc.tile_pool(name="ps_bc", bufs=1, space="PSUM"))
            ps_o = ph.enter_context(
                tc.tile_pool(name="ps_o", bufs=2, space="PSUM"))

            for hp in range(H // 2):
                pt = hp
                if 0 < hp < 7:
                    emit_kT(hp + 1, 0)
                    emit_kT(hp + 1, 1)
                if hp < 4:
                    emit_v(1, 2 * hp)
                    emit_v(1, 2 * hp + 1)
                if hp == 5:
                    prefetch_wo()
                et = {0: [], 1: []}
                for hi in range(2):
                    hh = hi * DH
                    for j in range(KT // 2):
                        p = ps_sim.tile([P, 2 * NQ], F32, tag="sim",
                                        name='sim')
                        for half in range(2):
                            kt = 2 * j + half
                            nc.tensor.matmul(
                                p[:, half * NQ:(half + 1) * NQ],
                                lhsT=_r(kTt[pt][hh:hh + DH,
                                                kt * P:(kt + 1) * P]),
                                rhs=_r(qTt[pt][hh:hh + DH, :]),
                                start=True, stop=True)
                        e = epool.tile([P, 2 * NQ], BF16, tag="e",
                                       name='e')
                        nc.scalar.activation(e[:], p[:], AF.Exp)
                        et[hi].append(e)
                pos = {}
                for hi in range(2):
                    pos[hi] = ps_o.tile([DH + 1, NQ], F32, tag="ov",
                                        name='ov')
                for kt in range(KT):
                    for hi in range(2):
                        h = 2 * hp + hi
                        esl = et[hi][kt // 2][:, (kt % 2) * NQ:
                                              (kt % 2 + 1) * NQ]
                        nc.tensor.matmul(
                            pos[hi][:],
                            lhsT=_r(vRt[kt][:, h * (DH + 1):
                                            (h + 1) * (DH + 1)]),
                            rhs=_r(esl),
                            start=(kt == 0), stop=(kt == KT - 1))
                for hi in range(2):
                    hh = hi * DH
                    po = pos[hi]
                    rf = spool.tile([DH + 1, NQ], F32, tag="rf", name='rf')
                    nc.vector.reciprocal_approx_fast(rf[:], po[:])
                    inv_s = spool.tile([1, NQ], BF16, tag="invs",
                                       name='invs')
                    nc.vector.tensor_copy(inv_s[:], rf[DH:DH + 1, :])
                    pb = ps_bc.tile([DH, NQ], F32, tag="bc", name='bc')
                    nc.tensor.matmul(pb[:], lhsT=_r(ones_row[0:1, 0:DH]),
                                     rhs=_r(inv_s[:]), start=True,
                                     stop=True)
                    binv = spool.tile([DH, NQ], F32, tag="binv",
                                      name='binv')
                    nc.vector.tensor_copy(binv[:], pb[:])
                    nc.vector.tensor_mul(outT[pt][hh:hh + DH, :],
                                         po[0:DH, :], binv[:])

        prj_cm.__exit__(None, None, None)
        wkv_cm.__exit__(None, None, None)
        qkv_cm.__exit__(None, None, None)
        hT_cm.__exit__(None, None, None)

        # prefetch all of W1 (64KB/part resident) while phase 4 runs
        w1_cm = tc.tile_pool(name="w1p", bufs=1)
        w1p = w1_cm.__enter__()
        w1t = [[w1p.tile([P, 2 * NQ], BF16, tag=f"w1_{g2}_{k}",
                         name=f"w1_{g2}_{k}") for k in range(KT)]
               for g2 in range(4)]
        for g2 in range(4):
            for k in range(KT):
                eng = nc.sync if k % 2 == 0 else nc.gpsimd
                eng.dma_start(w1t[g2][k][:],
                              W1[k * P:(k + 1) * P, g2 * D:(g2 + 1) * D])

        # ---------------- phase 4: Wo + residual + ln2 ----------------
        x1p = root.enter_context(tc.tile_pool(name="x1p", bufs=1, side='right'))
        x1t = [x1p.tile([P, NQ], BF16, tag=f"x1{k}", name=f"x1{k}")
               for k in range(KT)]
        h2p = root.enter_context(tc.tile_pool(name="h2p", bufs=1, side='right'))
        h2t = [h2p.tile([P, NQ], BF16, tag=f"h2{k}", name=f"h2{k}")
               for k in range(KT)]

        with ExitStack() as ph:
            rows4 = ph.enter_context(tc.tile_pool(name="p4rows", bufs=1))
            xpool = ph.enter_context(tc.tile_pool(name="p4x", bufs=1))
            tpool = ph.enter_context(tc.tile_pool(name="p4t", bufs=3))

            xo = [xpool.tile([P, NQ], BF16, tag=f"xo{k}", name=f"xo{k}")
                  for k in range(KT)]
            for k in range(KT):
                eng = nc.sync if k % 2 == 0 else nc.gpsimd
                eng.dma_start(xo[k][:], xT[k * P:(k + 1) * P, 0:NQ])

            bop_row = rows4.tile([1, D], BF16, name='bop_row')
            boT = const.tile([P, KT], F32, name='boT')
            gboT = const.tile([P, KT], F32, name='gboT')

            with ExitStack() as sec:
                psv = sec.enter_context(
                    tc.tile_pool(name="psv", bufs=2, space="PSUM"))
                pscol2 = sec.enter_context(
                    tc.tile_pool(name="pscol2", bufs=1, space="PSUM"))
                psy = sec.enter_context(
                    tc.tile_pool(name="psy", bufs=2, space="PSUM"))

                for g in range(2):
                    wch = wot[g]
                    sp = 32 * (g + 1)
                    mp = psv.tile([P, NQ], F32, tag="bvps", name='bvps')
                    for k in range(KT):
                        nc.tensor.matmul(mp[sp:sp + 1, :],
                                         lhsT=_r(bvT[:, k:k + 1]),
                                         rhs=_r(wch[k][:]),
                                         start=(k == 0), stop=(k == KT - 1))
                    nc.vector.tensor_add(
                        bop_row[0:1, g * NQ:(g + 1) * NQ], mp[sp:sp + 1, :],
                        bo_row[0:1, g * NQ:(g + 1) * NQ])
                    cols_from_row(bop_row[0:1, g * NQ:(g + 1) * NQ],
                                  boT[:, g * 4:(g + 1) * 4], pscol2)
                    nc.vector.tensor_mul(gboT[:, g * 4:(g + 1) * 4],
                                         gmsaT[:, g * 4:(g + 1) * 4],
                                         boT[:, g * 4:(g + 1) * 4])
                    for dot in range(4):
                        t = 4 * g + dot
                        p = psy.tile([P, NQ], F32, tag="y1", name='y1')
                        for k in range(KT):
                            nc.tensor.matmul(
                                p[:],
                                lhsT=_r(wch[k][:, dot * P:(dot + 1) * P]),
                                rhs=_r(outT[k][:]),
                                start=(k == 0), stop=(k == KT - 1))
                        nc.vector.affine_then_add(
                            x1t[t][:], p[:], xo[t][:],
                            scale=gmsaT[:, t:t + 1],
                            bias=gboT[:, t:t + 1])

            with ExitStack() as sec:
                psstat2 = sec.enter_context(
                    tc.tile_pool(name="psstat2", bufs=1, space="PSUM"))
                psbr2 = sec.enter_context(
                    tc.tile_pool(name="psbr2", bufs=1, space="PSUM"))

                ss = psstat2.tile([P, NQ], F32, tag="st2s", name='st2s')
                sq_ps = psstat2.tile([P, NQ], F32, tag="st2q", name='st2q')
                for k in range(KT):
                    sq = tpool.tile([P, NQ], BF16, tag="x1sq", name='x1sq')
                    nc.scalar.square(sq[:], x1t[k][:])
                    nc.tensor.matmul(ss[32:33, :], lhsT=_r(ones_col[:]),
                                     rhs=_r(x1t[k][:]),
                                     start=(k == 0), stop=(k == KT - 1))
                    nc.tensor.matmul(sq_ps[64:65, :], lhsT=_r(ones_col[:]),
                                     rhs=_r(sq[:]),
                                     start=(k == 0), stop=(k == KT - 1))
                s2row = rows4.tile([1, NQ], F32, name='s2row')
                nc.vector.tensor_copy(s2row[:], ss[32:33, :])
                sq2 = rows4.tile([1, NQ], F32, name='sq2')
                nc.vector.tensor_mul(sq2[:], s2row[:], s2row[:])
                t2row = rows4.tile([1, NQ], F32, name='t2row')
                nc.vector.scalar_tensor_tensor(
                    t2row[:], sq_ps[64:65, :], float(D), sq2[:],
                    ALU.mult, ALU.subtract)
                a2 = rows4.tile([1, NQ], BF16, name='a2')
                nc.scalar.activation(a2[:], t2row[:],
                                     AF.Abs_reciprocal_sqrt, bias=epsD[:])
                b2r = rows4.tile([1, NQ], BF16, name='b2r')
                nc.vector.scalar_tensor_tensor(
                    b2r[:], s2row[:], -1.0, a2[:], ALU.mult, ALU.mult)

                ba2p = psbr2.tile([P, NQ], F32, tag="ba2", name='ba2')
                nc.tensor.matmul(ba2p[:], lhsT=_r(dall_row[:]),
                                 rhs=_r(a2[:]), start=True, stop=True)
                ba2 = rows4.tile([P, NQ], BF16, name='ba2s')
                nc.vector.tensor_copy(ba2[:], ba2p[:])
                bb2p = psbr2.tile([P, NQ], F32, tag="bb2", name='bb2')
                nc.tensor.matmul(bb2p[:], lhsT=_r(ones_row[0:1, 0:P]),
                                 rhs=_r(b2r[:]), start=True, stop=True)
                bb2 = rows4.tile([P, NQ], BF16, name='bb2s')
                nc.vector.tensor_copy(bb2[:], bb2p[:])
                for k in range(KT):
                    t1 = tpool.tile([P, NQ], BF16, tag="t12", name='t12')
                    nc.vector.scalar_tensor_tensor(
                        t1[:], x1t[k][:], S2T[:, k:k + 1], ba2[:],
                        ALU.mult, ALU.mult)
                    nc.vector.affine_then_add(
                        h2t[k][:], bb2[:], t1[:],
                        scale=S2T[:, k:k + 1], bias=sh2T[:, k:k + 1])

        # ---------------- phase 5: MLP ----------------
        with ExitStack() as ph:
            gp = ph.enter_context(tc.tile_pool(name="gp", bufs=1))
            gTt = [gp.tile([P, NQ], BF16, tag=f"g{m}", name=f"g{m}")
                   for m in range(MT)]
            w2pool = ph.enter_context(tc.tile_pool(name="p5w2", bufs=16))
            opool = ph.enter_context(tc.tile_pool(name="p5o", bufs=3))
            ps1 = ph.enter_context(
                tc.tile_pool(name="ps1", bufs=4, space="PSUM"))
            ps2 = ph.enter_context(
                tc.tile_pool(name="ps2", bufs=1, space="PSUM"))

            g2b2T = const.tile([P, KT], F32, name='g2b2T')
            nc.vector.tensor_mul(g2b2T[:], gmlpT[:], b2T[:])

            for g2 in range(4):           # 4 column groups of 1024
                for dot in range(8):
                    m = 8 * g2 + dot
                    p = ps1.tile([P, NQ], F32, tag="m1", name='m1')
                    for k in range(KT):
                        nc.tensor.matmul(
                            p[:],
                            lhsT=_r(w1t[g2][k][:, dot * P:(dot + 1) * P]),
                            rhs=_r(h2t[k][:]),
                            start=(k == 0), stop=(k == KT - 1))
                    nc.scalar.activation(gTt[m][:], p[:], AF.Gelu_apprx_tanh,
                                         bias=b1T[:, m:m + 1])

            for half in range(2):
                pacc = [ps2.tile([P, NQ], F32, tag=f"acc{d}",
                                 name=f"acc{d}") for d in range(4)]
                for mk in range(MT):
                    w2c = w2pool.tile([P, NQ], BF16, tag="w2", name='w2')
                    eng = nc.sync if mk % 2 == 0 else nc.gpsimd
                    eng.dma_start(
                        w2c[:], W2[mk * P:(mk + 1) * P,
                                   half * NQ:(half + 1) * NQ])
                    for d in range(4):
                        nc.tensor.matmul(
                            pacc[d][:],
                            lhsT=_r(w2c[:, d * P:(d + 1) * P]),
                            rhs=_r(gTt[mk][:]),
                            start=(mk == 0), stop=(mk == MT - 1))
                for d in range(4):
                    t = half * 4 + d
                    yt = opool.tile([P, NQ], BF16, tag="yout", name='yout')
                    nc.vector.affine_then_add(
                        yt[:], pacc[d][:], x1t[t][:],
                        scale=gmlpT[:, t:t + 1], bias=g2b2T[:, t:t + 1])
                    nc.scalar.dma_start(yT[t * P:(t + 1) * P, :], yt[:])

        w1_cm.__exit__(None, None, None)
        wo_cm.__exit__(None, None, None)
        op_cm.__exit__(None, None, None)

    nc.compile()
    return nc


_NC = None


def _get_nc():
    global _NC
    if _NC is None:
        _NC = build()
    return _NC


def _prep_inputs(x, c, Wq, bq, Wkv, bkv, Wo, bo, W1, b1, W2, b2, Wada, bada):
    import ml_dtypes
    f = np.float32
    bf = ml_dtypes.bfloat16
    col = lambda v, n: np.ascontiguousarray(
        np.asarray(v, f).reshape(n, P).T)
    shared = {
        "Wq": np.asarray(Wq, f).astype(bf), "Wkv": np.asarray(Wkv, f).astype(bf),
        "Wo": np.asarray(Wo, f).astype(bf), "W1": np.asarray(W1, f).astype(bf),
        "W2": np.asarray(W2, f).astype(bf), "Wada": np.asarray(Wada, f).astype(bf),
        "bada_r": np.asarray(bada, f).reshape(1, -1),
        "bq_c": col(bq, KT), "bk_c": col(np.asarray(bkv, f)[:D], KT),
        "bv_c": col(np.asarray(bkv, f)[D:], KT).astype(bf),
        "bo_r": np.asarray(bo, f).reshape(1, -1),
        "b1_c": col(b1, MT), "b2_c": col(b2, KT),
    }
    in_maps = []
    for core in range(NCORES):
        b, half = core // 2, core % 2
        xb = np.asarray(x[b], f)
        perm = np.concatenate(
            [xb[half * NQ:(half + 1) * NQ],
             xb[(1 - half) * NQ:(2 - half) * NQ]], axis=0)
        m = dict(shared)
        m["xT"] = np.ascontiguousarray(perm.T).astype(bf)
        m["crow"] = np.asarray(c[b:b + 1], f)
        in_maps.append(m)
    return in_maps


def _run(inputs, trace=False):
    nc = _get_nc()
    in_maps = _prep_inputs(**inputs)
    res = run_bass_kernel_spmd(nc, in_maps, core_ids=list(range(NCORES)),
                               trace=trace)
    B = 4
    y = np.empty((B, N, D), np.float32)
    for core in range(NCORES):
        b, half = core // 2, core % 2
        y[b, half * NQ:(half + 1) * NQ, :] = (
            res.results[core]["yT"].astype(np.float32).T)
    return y, res


def kernel(**inputs):
    y, _ = _run(inputs, trace=False)
    return y



# revision 49
# speedup vs baseline: 1.1569x; 1.1569x over previous
"""AdaLN attention block (DiT-style) on 8 TRN2 NeuronCores.

Sharding: 8 cores = 4 batches x 2 token-halves, no collectives. Core c handles
batch c//2 and query-token half c%2: layernorm1 and k/v are computed over the
full (permuted) sequence, everything else only for the own 512 query rows.

Device layout is feature-major (activations transposed, [d, n]). X @ W runs
with W column-tiles stationary and X^T moving, producing Y^T directly.
LayerNorm statistics use ones-vector matmuls (partition-axis sums on the PE);
the AdaLN modulate is h = x*A + B with rank-1 A/B built by K=1 outer-product
matmuls into PSUM. Softmax skips max-subtraction (fp32 exp is safe for this
distribution); the denominator is a ones-column appended to the attn@v
stationary operand; normalization is folded in per head via a broadcast
reciprocal.
"""

import numpy as np
from contextlib import ExitStack

import concourse.bass as bass
import concourse.bacc as bacc
import concourse.mybir as mybir
from concourse import tile
from concourse.bass_utils import run_bass_kernel_spmd

P = 128
D = 1024
N = 1024
NQ = 512
H = 16
DH = 64
MLPD = 4096
EPS = 1e-6
NCORES = 8

F32 = mybir.dt.float32
BF16 = mybir.dt.bfloat16
AF = mybir.ActivationFunctionType
ALU = mybir.AluOpType

KT = D // P           # 8 contraction tiles over D
MT = MLPD // P        # 32 tiles over MLP dim


def _r(ap):
    return ap


def build():
    nc = bacc.Bacc("TRN2", target_bir_lowering=False, debug=False,
                   num_devices=NCORES)

    xT = nc.dram_tensor("xT", [D, N], BF16, kind="ExternalInput")
    crow = nc.dram_tensor("crow", [1, D], F32, kind="ExternalInput")
    Wq = nc.dram_tensor("Wq", [D, D], BF16, kind="ExternalInput")
    Wkv = nc.dram_tensor("Wkv", [D, 2 * D], BF16, kind="ExternalInput")
    Wo = nc.dram_tensor("Wo", [D, D], BF16, kind="ExternalInput")
    W1 = nc.dram_tensor("W1", [D, MLPD], BF16, kind="ExternalInput")
    W2 = nc.dram_tensor("W2", [MLPD, D], BF16, kind="ExternalInput")
    Wada = nc.dram_tensor("Wada", [D, 6 * D], BF16, kind="ExternalInput")
    bada_r = nc.dram_tensor("bada_r", [1, 6 * D], F32, kind="ExternalInput")
    bq_c = nc.dram_tensor("bq_c", [P, KT], F32, kind="ExternalInput")
    bk_c = nc.dram_tensor("bk_c", [P, KT], F32, kind="ExternalInput")
    bv_c = nc.dram_tensor("bv_c", [P, KT], BF16, kind="ExternalInput")
    bo_r = nc.dram_tensor("bo_r", [1, D], F32, kind="ExternalInput")
    b1_c = nc.dram_tensor("b1_c", [P, MT], F32, kind="ExternalInput")
    b2_c = nc.dram_tensor("b2_c", [P, KT], F32, kind="ExternalInput")
    yT = nc.dram_tensor("yT", [D, NQ], BF16, kind="ExternalOutput")

    with tile.TileContext(nc) as tc, ExitStack() as root:
        const = root.enter_context(tc.tile_pool(name="const", bufs=1))
        rootrows = root.enter_context(tc.tile_pool(name="rootrows", bufs=1))
        mod_row = rootrows.tile([1, 6 * D], BF16, name='mod_row')
        bad_full = rootrows.tile([1, 6 * D], F32, name='bad_full')

        ones_col = const.tile([P, 1], BF16, name='ones_col')
        nc.vector.memset(ones_col[:], 1.0)
        ones_row = const.tile([1, NQ], BF16, name='ones_row')
        nc.vector.memset(ones_row[:], 1.0)
        dall_row = const.tile([1, P], BF16, name='dall_row')
        nc.vector.memset(dall_row[:], float(D))
        epsD = const.tile([1, 1], F32, name='epsD')
        nc.vector.memset(epsD[:], EPS * D * D)

        # bias columns are loaded on the gpsimd queue AFTER the x tiles so
        # the latency-critical transfers go first
        bqT = const.tile([P, KT], F32, name='bqT')
        bkT = const.tile([P, KT], F32, name='bkT')
        bvT = const.tile([P, KT], BF16, name='bvT')
        b1T = const.tile([P, MT], F32, name='b1T')
        b2T = const.tile([P, KT], F32, name='b2T')
        bo_row = const.tile([1, D], F32, name='bo_row')
        bqT_s = const.tile([P, KT], F32, name='bqT_s')

        def load_consts():
            nc.gpsimd.dma_start(bqT[:], bq_c[:])
            nc.gpsimd.dma_start(bkT[:], bk_c[:])
            nc.gpsimd.dma_start(bvT[:], bv_c[:])
            nc.gpsimd.dma_start(b1T[:], b1_c[:])
            nc.gpsimd.dma_start(b2T[:], b2_c[:])
            nc.gpsimd.dma_start(bo_row[:], bo_r[:])
            nc.vector.tensor_scalar_mul(bqT_s[:], bqT[:], DH ** -0.5)

        def cols_from_row(row_ap, dst, psum_pool, plus1=False):
            """[1, n*128] row -> [128, n] column tile via K=1 matmuls."""
            n = dst.shape[-1]
            ps = psum_pool.tile([P, n], F32, tag="colps", name='colps')
            for j in range(n):
                nc.tensor.matmul(ps[:, j:j + 1],
                                 lhsT=_r(row_ap[0:1, j * P:(j + 1) * P]),
                                 rhs=_r(ones_row[0:1, 0:1]),
                                 start=True, stop=True)
            if plus1:
                nc.vector.tensor_scalar_add(dst[:], ps[:], 1.0)
            else:
                nc.vector.tensor_copy(dst[:], ps[:])
            return dst

        csT = const.tile([P, KT], BF16, name='csT')
        gmsaT = const.tile([P, KT], F32, name='gmsaT')
        gmlpT = const.tile([P, KT], F32, name='gmlpT')
        S1T = const.tile([P, KT], F32, name='S1T')
        sh1T = const.tile([P, KT], F32, name='sh1T')
        S2T = const.tile([P, KT], F32, name='S2T')
        sh2T = const.tile([P, KT], F32, name='sh2T')

        # persistent activation arrays (distinct tag per tile, 1 buf each)
        op_cm = tc.tile_pool(name="op", bufs=1, side='left')
        op_ = op_cm.__enter__()
        outT = [op_.tile([P, NQ], BF16, tag=f"o{k}", name=f"o{k}")
                for k in range(KT)]
        wo_cm = tc.tile_pool(name="wop", bufs=1, side='left')
        wop = wo_cm.__enter__()
        wot = [[wop.tile([P, NQ], BF16, tag=f"wo{g}_{k}", name=f"wo{g}_{k}")
                for k in range(KT)] for g in range(2)]
        hT_cm = tc.tile_pool(name="hTp", bufs=1, side='left')
        hTp = hT_cm.__enter__()
        hT = [hTp.tile([P, N], BF16, tag=f"h{k}", name=f"h{k}")
              for k in range(KT)]

        # ---------------- phase 0+1: mod vector & ln1 ----------------
        with ExitStack() as ph:
            rows = ph.enter_context(tc.tile_pool(name="p0rows", bufs=1))
            xpool = ph.enter_context(tc.tile_pool(name="p0x", bufs=1))
            sqpool = ph.enter_context(tc.tile_pool(name="p0sq", bufs=4))
            wpool = ph.enter_context(tc.tile_pool(name="p0w", bufs=8))

            # c + bada first, on the scalar HWDGE queue so they are not
            # stuck behind the bulk x/Wada transfers
            c_sb = rows.tile([1, D], F32, name='c_sb')
            nc.scalar.dma_start(c_sb[:], crow[:])
            nc.scalar.dma_start(bad_full[:], bada_r[:])

            xt = [xpool.tile([P, N], BF16, tag=f"x{k}", name=f"x{k}")
                  for k in range(KT)]
            for k in range(KT):
                eng = nc.sync if k % 2 == 0 else nc.gpsimd
                eng.dma_start(xt[k][:], xT[k * P:(k + 1) * P, :])
            load_consts()

            with ExitStack() as sec:
                pscol = sec.enter_context(
                    tc.tile_pool(name="pscol", bufs=1, space="PSUM"))
                psmod = sec.enter_context(
                    tc.tile_pool(name="psmod", bufs=3, space="PSUM"))
                psstat = sec.enter_context(
                    tc.tile_pool(name="psstat", bufs=2, space="PSUM"))

                # silu(c) and its column layout
                cs_row = rows.tile([1, D], BF16, name='cs_row')
                nc.scalar.activation(cs_row[:], c_sb[:], AF.Silu)
                cols_from_row(cs_row, csT, pscol)

                # ln1 stats: per 512-chunk, sum and sumsq over d.
                # sum chains land on col-strip 1 (partition 32), sumsq on
                # strip 2 (partition 64): the PE runs them concurrently.
                s_row = rows.tile([1, N], F32, name='s_row')
                t_row = rows.tile([1, N], F32, name='t_row')
                for ch in range(2):
                    sl = slice(ch * NQ, (ch + 1) * NQ)
                    ss = psstat.tile([P, NQ], F32, tag="st_s", name='st_s')
                    sq_ps = psstat.tile([P, NQ], F32, tag="st_q",
                                        name='st_q')
                    for k in range(KT):
                        sq = sqpool.tile([P, NQ], BF16, tag="xsq",
                                         name='xsq')
                        nc.scalar.square(sq[:], xt[k][:, sl])
                        nc.tensor.matmul(ss[32:33, :],
                                         lhsT=_r(ones_col[:]),
                                         rhs=_r(xt[k][:, sl]),
                                         start=(k == 0), stop=(k == KT - 1))
                        nc.tensor.matmul(sq_ps[64:65, :],
                                         lhsT=_r(ones_col[:]),
                                         rhs=_r(sq[:]),
                                         start=(k == 0), stop=(k == KT - 1))
                    nc.vector.tensor_copy(s_row[0:1, sl], ss[32:33, :])
                    # t = D*sumsq - sum^2  (var*D^2, istd = D/sqrt(t+eps*D^2))
                    s2 = rows.tile([1, NQ], F32, name=f's2_{ch}')
                    nc.vector.tensor_mul(s2[:], s_row[0:1, sl],
                                         s_row[0:1, sl])
                    nc.vector.scalar_tensor_tensor(
                        t_row[0:1, sl], sq_ps[64:65, :], float(D), s2[:],
                        ALU.mult, ALU.subtract)

                # mod = silu(c) @ Wada + bada — groups 0-3 (sh/sc_msa) now;
                # groups 4-11 are deferred into phase 2 so their Wada DMA
                # doesn't block the PE stream here. Groups rotate over
                # col-strips 0/1/2 for 3x concurrency.
                wch0 = [wpool.tile([P, 4 * NQ], BF16, tag="wada",
                                   name='wada') for _ in range(KT)]
                for k in range(KT):
                    eng = (nc.sync, nc.gpsimd, nc.scalar)[k % 3]
                    eng.dma_start(wch0[k][:],
                                  Wada[k * P:(k + 1) * P, 0:2 * D])
                for g in range(4):
                    sp = 32 * (g % 3)
                    mp = psmod.tile([P, NQ], F32, tag="modps",
                                    name='modps')
                    for k in range(KT):
                        nc.tensor.matmul(
                            mp[sp:sp + 1, :],
                            lhsT=_r(csT[:, k:k + 1]),
                            rhs=_r(wch0[k][:, g * NQ:(g + 1) * NQ]),
                            start=(k == 0), stop=(k == KT - 1))
                    nc.vector.tensor_add(
                        mod_row[0:1, g * NQ:(g + 1) * NQ],
                        mp[sp:sp + 1, :],
                        bad_full[0:1, g * NQ:(g + 1) * NQ])

                # r = 1/sqrt(t + eps*D^2); istd = D*r (D folded into the
                # broadcast lhsT); b = -mu*istd = -sum*r
                a_row = rows.tile([1, N], BF16, name='a_row')
                nc.scalar.activation(a_row[:], t_row[:],
                                     AF.Abs_reciprocal_sqrt, bias=epsD[:])
                b_row = rows.tile([1, N], BF16, name='b_row')
                nc.vector.scalar_tensor_tensor(
                    b_row[:], s_row[:], -1.0, a_row[:],
                    ALU.mult, ALU.mult)

                # modulation columns for the attention branch
                cols_from_row(mod_row[0:1, 0:D], sh1T, pscol)
                cols_from_row(mod_row[0:1, D:2 * D], S1T, pscol,
                              plus1=True)

            # h = (x*S1*a_bcast) + (b_bcast*S1 + sh1): two fused DVE ops
            # per chunk against shared rank-1 broadcast tiles.
            with ExitStack() as sec:
                psbr = sec.enter_context(
                    tc.tile_pool(name="psbr", bufs=1, space="PSUM"))
                brpool = sec.enter_context(tc.tile_pool(name="brp",
                                                        bufs=1))
                ba = {}
                bb = {}
                for ch in range(2):
                    sl = slice(ch * NQ, (ch + 1) * NQ)
                    bap = psbr.tile([P, NQ], F32, tag=f"ba{ch}",
                                    name=f"ba{ch}")
                    nc.tensor.matmul(bap[:], lhsT=_r(dall_row[:]),
                                     rhs=_r(a_row[0:1, sl]),
                                     start=True, stop=True)
                    ba[ch] = brpool.tile([P, NQ], BF16, tag=f"bas{ch}",
                                         name=f"bas{ch}")
                    nc.vector.tensor_copy(ba[ch][:], bap[:])
                    bbp = psbr.tile([P, NQ], F32, tag=f"bb{ch}",
                                    name=f"bb{ch}")
                    nc.tensor.matmul(bbp[:], lhsT=_r(ones_row[0:1, 0:P]),
                                     rhs=_r(b_row[0:1, sl]),
                                     start=True, stop=True)
                    bb[ch] = brpool.tile([P, NQ], BF16, tag=f"bbs{ch}",
                                         name=f"bbs{ch}")
                    nc.vector.tensor_copy(bb[ch][:], bbp[:])
                for ch in range(2):
                    for k in range(KT):
                        sl = slice(ch * NQ, (ch + 1) * NQ)
                        t1 = sqpool.tile([P, NQ], BF16, tag="t1",
                                         name='t1')
                        nc.vector.scalar_tensor_tensor(
                            t1[:], xt[k][:, sl], S1T[:, k:k + 1],
                            ba[ch][:], ALU.mult, ALU.mult)
                        nc.vector.affine_then_add(
                            hT[k][:, sl], bb[ch][:], t1[:],
                            scale=S1T[:, k:k + 1], bias=sh1T[:, k:k + 1])

        # ---------------- phase 2: q, k, v projections ----------------
        qkv_cm = tc.tile_pool(name="qkvp", bufs=1, side='right')
        qkvp = qkv_cm.__enter__()
        qTt = [qkvp.tile([P, NQ], BF16, tag=f"q{k}", name=f"q{k}")
               for k in range(KT)]
        kTt = [qkvp.tile([P, N], BF16, tag=f"k{k}", name=f"k{k}")
               for k in range(KT)]
        vRt = [qkvp.tile([P, H * (DH + 1)], BF16, tag=f"v{k}", name=f"v{k}")
               for k in range(KT)]

        wkv_cm = tc.tile_pool(name="wkvp", bufs=1, side='right')
        wkvp = wkv_cm.__enter__()
        wkc = {}   # (g) -> k-part chunks; ('v', vg) -> v-part chunks
        for g in range(2):
            wkc[g] = [wkvp.tile([P, NQ], BF16, tag=f"kg{g}_{k}",
                                name=f"kg{g}_{k}") for k in range(KT)]
            for k in range(KT):
                eng = (nc.sync, nc.gpsimd, nc.scalar)[k % 3]
                eng.dma_start(wkc[g][k][:],
                              Wkv[k * P:(k + 1) * P, g * NQ:(g + 1) * NQ])
        for vg in range(2):
            wkc['v', vg] = [wkvp.tile([P, NQ], BF16, tag=f"vg{vg}_{k}",
                                      name=f"vg{vg}_{k}")
                            for k in range(KT)]
            for k in range(KT):
                eng = (nc.sync, nc.gpsimd, nc.scalar)[k % 3]
                eng.dma_start(wkc['v', vg][k][:],
                              Wkv[k * P:(k + 1) * P,
                                  D + vg * NQ:D + (vg + 1) * NQ])

        prj_cm = tc.tile_pool(name="prjps", bufs=1, space="PSUM",
                              side='right')
        prjps = prj_cm.__enter__()

        def emit_kT(t, ch):
            g, dot = t // 4, t % 4
            sl = slice(ch * NQ, (ch + 1) * NQ)
            p = prjps.tile([P, NQ], F32, tag="prj", name='prj')
            for k in range(KT):
                nc.tensor.matmul(
                    p[:], lhsT=_r(wkc[g][k][:, dot * P:(dot + 1) * P]),
                    rhs=_r(hT[k][:, sl]),
                    start=(k == 0), stop=(k == KT - 1))
            nc.vector.tensor_scalar_add(kTt[t][:, sl], p[:],
                                        bkT[:, t:t + 1])

        def emit_v(vg, nt):
            p = prjps.tile([P, NQ], F32, tag="prj", name='prj')
            for k in range(KT):
                nc.tensor.matmul(
                    p[:], lhsT=_r(hT[k][:, nt * P:(nt + 1) * P]),
                    rhs=_r(wkc['v', vg][k][:]),
                    start=(k == 0), stop=(k == KT - 1))
            vv = vRt[nt].rearrange("p (h w) -> p h w", w=DH + 1)
            pv = p.rearrange("p (h w) -> p h w", w=DH)
            nc.vector.tensor_copy(vv[:, vg * 8:(vg + 1) * 8, 0:DH], pv[:])

        with ExitStack() as ph:
            wpool = ph.enter_context(tc.tile_pool(name="p2w", bufs=26))
            wadap = ph.enter_context(tc.tile_pool(name="p2wada", bufs=8))
            ps = ph.enter_context(
                tc.tile_pool(name="p2ps", bufs=3, space="PSUM"))
            psmod2 = ph.enter_context(
                tc.tile_pool(name="psmod2", bufs=3, space="PSUM"))
            pscolb = ph.enter_context(
                tc.tile_pool(name="pscolb", bufs=1, space="PSUM"))

            for nt in range(KT):
                vv = vRt[nt].rearrange("p (h w) -> p h w", w=DH + 1)
                nc.vector.memset(vv[:, :, DH:DH + 1], 1.0)

            def stationary_group(wdram, col0, movs, evict, tagp):
                wch = [wpool.tile([P, NQ], BF16, tag=tagp, name=tagp)
                       for _ in range(KT)]
                for k in range(KT):
                    eng = (nc.sync, nc.gpsimd, nc.scalar)[k % 3]
                    eng.dma_start(
                        wch[k][:], wdram[k * P:(k + 1) * P, col0:col0 + NQ])
                for dot in range(4):
                    p = ps.tile([P, NQ], F32, tag="prj", name='prj')
                    for k in range(KT):
                        nc.tensor.matmul(
                            p[:], lhsT=_r(wch[k][:, dot * P:(dot + 1) * P]),
                            rhs=movs[k], start=(k == 0), stop=(k == KT - 1))
                    evict(dot, p)

            # q^T (own rows), scaled by 1/sqrt(DH)
            for g in range(2):
                def ev_q(dot, p, g=g):
                    t = 4 * g + dot
                    nc.vector.tensor_scalar(qTt[t][:], p[:], DH ** -0.5,
                                            bqT_s[:, t:t + 1],
                                            ALU.mult, ALU.add)
                stationary_group(Wq, g * NQ,
                                 [_r(hT[k][:, 0:NQ]) for k in range(KT)],
                                 ev_q, "wst")

            # k^T tiles 0-1 and v-group 0 now; the rest is emitted inside
            # the attention loop as just-in-time full-array work that keeps
            # the PE clock warm
            for t in range(2):
                emit_kT(t, 0)
                emit_kT(t, 1)
            for nt in range(KT):
                emit_v(0, nt)

            # deferred mod groups 4-11 (msa gate + mlp modulation): their
            # Wada traffic and PE work hide behind the projection phase
            for g2 in range(2, 6):
                wch = [wadap.tile([P, 2 * NQ], BF16, tag="wada2",
                                  name='wada2') for _ in range(KT)]
                for k in range(KT):
                    eng = (nc.sync, nc.gpsimd, nc.scalar)[k % 3]
                    eng.dma_start(
                        wch[k][:], Wada[k * P:(k + 1) * P,
                                        g2 * D:(g2 + 1) * D])
                for sub in range(2):
                    g = 2 * g2 + sub
                    sp = 32 * (g % 3)
                    mp = psmod2.tile([P, NQ], F32, tag="modps2",
                                     name='modps2')
                    for k in range(KT):
                        nc.tensor.matmul(
                            mp[sp:sp + 1, :],
                            lhsT=_r(csT[:, k:k + 1]),
                            rhs=_r(wch[k][:, sub * NQ:(sub + 1) * NQ]),
                            start=(k == 0), stop=(k == KT - 1))
                    nc.vector.tensor_add(
                        mod_row[0:1, g * NQ:(g + 1) * NQ],
                        mp[sp:sp + 1, :],
                        bad_full[0:1, g * NQ:(g + 1) * NQ])
            cols_from_row(mod_row[0:1, 2 * D:3 * D], gmsaT, pscolb)
            cols_from_row(mod_row[0:1, 3 * D:4 * D], sh2T, pscolb)
            cols_from_row(mod_row[0:1, 4 * D:5 * D], S2T, pscolb,
                          plus1=True)
            cols_from_row(mod_row[0:1, 5 * D:6 * D], gmlpT, pscolb)


        # ---------------- phase 3: attention ----------------

        def prefetch_wo():
            for g in range(2):
                for k in range(KT):
                    eng = nc.sync if k % 2 == 0 else nc.gpsimd
                    eng.dma_start(
                        wot[g][k][:],
                        Wo[k * P:(k + 1) * P, g * NQ:(g + 1) * NQ])

        with ExitStack() as ph:
            epool = ph.enter_context(tc.tile_pool(name="p3e", bufs=10))
            spool = ph.enter_context(tc.tile_pool(name="p3s", bufs=3))
            ps_sim = ph.enter_context(
                tc.tile_pool(name="ps_sim", bufs=2, space="PSUM"))
            ps_bc = ph.enter_context(
                tc.tile_pool(name="ps_bc", bufs=1, space="PSUM"))
            ps_o = ph.enter_context(
                tc.tile_pool(name="ps_o", bufs=2, space="PSUM"))

            for hp in range(H // 2):
                pt = hp
                if 0 < hp < 7:
                    emit_kT(hp + 1, 0)
                    emit_kT(hp + 1, 1)
                if hp < 4:
                    emit_v(1, 2 * hp)
                    emit_v(1, 2 * hp + 1)
                if hp == 5:
                    prefetch_wo()
                et = {0: [], 1: []}
                for hi in range(2):
                    hh = hi * DH
                    for j in range(KT // 2):
                        p = ps_sim.tile([P, 2 * NQ], F32, tag="sim",
                                        name='sim')
                        for half in range(2):
                            kt = 2 * j + half
                            nc.tensor.matmul(
                                p[:, half * NQ:(half + 1) * NQ],
                                lhsT=_r(kTt[pt][hh:hh + DH,
                                                kt * P:(kt + 1) * P]),
                                rhs=_r(qTt[pt][hh:hh + DH, :]),
                                start=True, stop=True)
                        e = epool.tile([P, 2 * NQ], BF16, tag="e",
                                       name='e')
                        nc.scalar.activation(e[:], p[:], AF.Exp)
                        et[hi].append(e)
                pos = {}
                for hi in range(2):
                    pos[hi] = ps_o.tile([DH + 1, NQ], F32, tag="ov",
                                        name='ov')
                for kt in range(KT):
                    for hi in range(2):
                        h = 2 * hp + hi
                        esl = et[hi][kt // 2][:, (kt % 2) * NQ:
                                              (kt % 2 + 1) * NQ]
                        nc.tensor.matmul(
                            pos[hi][:],
                            lhsT=_r(vRt[kt][:, h * (DH + 1):
                                            (h + 1) * (DH + 1)]),
                            rhs=_r(esl),
                            start=(kt == 0), stop=(kt == KT - 1))
                for hi in range(2):
                    hh = hi * DH
                    po = pos[hi]
                    rf = spool.tile([DH + 1, NQ], F32, tag="rf", name='rf')
                    nc.vector.reciprocal_approx_fast(rf[:], po[:])
                    inv_s = spool.tile([1, NQ], BF16, tag="invs",
                                       name='invs')
                    nc.vector.tensor_copy(inv_s[:], rf[DH:DH + 1, :])
                    pb = ps_bc.tile([DH, NQ], F32, tag="bc", name='bc')
                    nc.tensor.matmul(pb[:], lhsT=_r(ones_row[0:1, 0:DH]),
                                     rhs=_r(inv_s[:]), start=True,
                                     stop=True)
                    binv = spool.tile([DH, NQ], F32, tag="binv",
                                      name='binv')
                    nc.vector.tensor_copy(binv[:], pb[:])
                    nc.vector.tensor_mul(outT[pt][hh:hh + DH, :],
                                         po[0:DH, :], binv[:])

        prj_cm.__exit__(None, None, None)
        wkv_cm.__exit__(None, None, None)
        qkv_cm.__exit__(None, None, None)
        hT_cm.__exit__(None, None, None)

        # prefetch all of W1 (64KB/part resident) while phase 4 runs
        w1_cm = tc.tile_pool(name="w1p", bufs=1)
        w1p = w1_cm.__enter__()
        w1t = [[w1p.tile([P, 2 * NQ], BF16, tag=f"w1_{g2}_{k}",
                         name=f"w1_{g2}_{k}") for k in range(KT)]
               for g2 in range(4)]
        for g2 in range(4):
            for k in range(KT):
                eng = nc.sync if k % 2 == 0 else nc.gpsimd
                eng.dma_start(w1t[g2][k][:],
                              W1[k * P:(k + 1) * P, g2 * D:(g2 + 1) * D])

        # ---------------- phase 4: Wo + residual + ln2 ----------------
        x1p = root.enter_context(tc.tile_pool(name="x1p", bufs=1, side='right'))
        x1t = [x1p.tile([P, NQ], BF16, tag=f"x1{k}", name=f"x1{k}")
               for k in range(KT)]
        h2p = root.enter_context(tc.tile_pool(name="h2p", bufs=1, side='right'))
        h2t = [h2p.tile([P, NQ], BF16, tag=f"h2{k}", name=f"h2{k}")
               for k in range(KT)]

        with ExitStack() as ph:
            rows4 = ph.enter_context(tc.tile_pool(name="p4rows", bufs=1))
            xpool = ph.enter_context(tc.tile_pool(name="p4x", bufs=1))
            tpool = ph.enter_context(tc.tile_pool(name="p4t", bufs=3))

            xo = [xpool.tile([P, NQ], BF16, tag=f"xo{k}", name=f"xo{k}")
                  for k in range(KT)]
            for k in range(KT):
                eng = nc.sync if k % 2 == 0 else nc.gpsimd
                eng.dma_start(xo[k][:], xT[k * P:(k + 1) * P, 0:NQ])

            bop_row = rows4.tile([1, D], BF16, name='bop_row')
            boT = const.tile([P, KT], F32, name='boT')
            gboT = const.tile([P, KT], F32, name='gboT')

            with ExitStack() as sec:
                psv = sec.enter_context(
                    tc.tile_pool(name="psv", bufs=2, space="PSUM"))
                pscol2 = sec.enter_context(
                    tc.tile_pool(name="pscol2", bufs=1, space="PSUM"))
                psy = sec.enter_context(
                    tc.tile_pool(name="psy", bufs=2, space="PSUM"))

                for g in range(2):
                    wch = wot[g]
                    sp = 32 * (g + 1)
                    mp = psv.tile([P, NQ], F32, tag="bvps", name='bvps')
                    for k in range(KT):
                        nc.tensor.matmul(mp[sp:sp + 1, :],
                                         lhsT=_r(bvT[:, k:k + 1]),
                                         rhs=_r(wch[k][:]),
                                         start=(k == 0), stop=(k == KT - 1))
                    nc.vector.tensor_add(
                        bop_row[0:1, g * NQ:(g + 1) * NQ], mp[sp:sp + 1, :],
                        bo_row[0:1, g * NQ:(g + 1) * NQ])
                    cols_from_row(bop_row[0:1, g * NQ:(g + 1) * NQ],
                                  boT[:, g * 4:(g + 1) * 4], pscol2)
                    nc.vector.tensor_mul(gboT[:, g * 4:(g + 1) * 4],
                                         gmsaT[:, g * 4:(g + 1) * 4],
                                         boT[:, g * 4:(g + 1) * 4])
                    for dot in range(4):
                        t = 4 * g + dot
                        p = psy.tile([P, NQ], F32, tag="y1", name='y1')
                        for k in range(KT):
                            nc.tensor.matmul(
                                p[:],
                                lhsT=_r(wch[k][:, dot * P:(dot + 1) * P]),
                                rhs=_r(outT[k][:]),
                                start=(k == 0), stop=(k == KT - 1))
                        nc.vector.affine_then_add(
                            x1t[t][:], p[:], xo[t][:],
                            scale=gmsaT[:, t:t + 1],
                            bias=gboT[:, t:t + 1])

            with ExitStack() as sec:
                psstat2 = sec.enter_context(
                    tc.tile_pool(name="psstat2", bufs=1, space="PSUM"))
                psbr2 = sec.enter_context(
                    tc.tile_pool(name="psbr2", bufs=1, space="PSUM"))

                ss = psstat2.tile([P, NQ], F32, tag="st2s", name='st2s')
                sq_ps = psstat2.tile([P, NQ], F32, tag="st2q", name='st2q')
                for k in range(KT):
                    sq = tpool.tile([P, NQ], BF16, tag="x1sq", name='x1sq')
                    nc.scalar.square(sq[:], x1t[k][:])
                    nc.tensor.matmul(ss[32:33, :], lhsT=_r(ones_col[:]),
                                     rhs=_r(x1t[k][:]),
                                     start=(k == 0), stop=(k == KT - 1))
                    nc.tensor.matmul(sq_ps[64:65, :], lhsT=_r(ones_col[:]),
                                     rhs=_r(sq[:]),
                                     start=(k == 0), stop=(k == KT - 1))
                s2row = rows4.tile([1, NQ], F32, name='s2row')
                nc.vector.tensor_copy(s2row[:], ss[32:33, :])
                sq2 = rows4.tile([1, NQ], F32, name='sq2')
                nc.vector.tensor_mul(sq2[:], s2row[:], s2row[:])
                t2row = rows4.tile([1, NQ], F32, name='t2row')
                nc.vector.scalar_tensor_tensor(
                    t2row[:], sq_ps[64:65, :], float(D), sq2[:],
                    ALU.mult, ALU.subtract)
                a2 = rows4.tile([1, NQ], BF16, name='a2')
                nc.scalar.activation(a2[:], t2row[:],
                                     AF.Abs_reciprocal_sqrt, bias=epsD[:])
                b2r = rows4.tile([1, NQ], BF16, name='b2r')
                nc.vector.scalar_tensor_tensor(
                    b2r[:], s2row[:], -1.0, a2[:], ALU.mult, ALU.mult)

                ba2p = psbr2.tile([P, NQ], F32, tag="ba2", name='ba2')
                nc.tensor.matmul(ba2p[:], lhsT=_r(dall_row[:]),
                                 rhs=_r(a2[:]), start=True, stop=True)
                ba2 = rows4.tile([P, NQ], BF16, name='ba2s')
                nc.vector.tensor_copy(ba2[:], ba2p[:])
                bb2p = psbr2.tile([P, NQ], F32, tag="bb2", name='bb2')
                nc.tensor.matmul(bb2p[:], lhsT=_r(ones_row[0:1, 0:P]),
                                 rhs=_r(b2r[:]), start=True, stop=True)
                bb2 = rows4.tile([P, NQ], BF16, name='bb2s')
                nc.vector.tensor_copy(bb2[:], bb2p[:])
                for k in range(KT):
                    t1 = tpool.tile([P, NQ], BF16, tag="t12", name='t12')
                    nc.vector.scalar_tensor_tensor(
                        t1[:], x1t[k][:], S2T[:, k:k + 1], ba2[:],
                        ALU.mult, ALU.mult)
                    nc.vector.affine_then_add(
                        h2t[k][:], bb2[:], t1[:],
                        scale=S2T[:, k:k + 1], bias=sh2T[:, k:k + 1])

        # ---------------- phase 5: MLP ----------------
        with ExitStack() as ph:
            gp = ph.enter_context(tc.tile_pool(name="gp", bufs=1))
            gTt = [gp.tile([P, NQ], BF16, tag=f"g{m}", name=f"g{m}")
                   for m in range(MT)]
            w2pool = ph.enter_context(tc.tile_pool(name="p5w2", bufs=16))
            opool = ph.enter_context(tc.tile_pool(name="p5o", bufs=3))
            ps1 = ph.enter_context(
                tc.tile_pool(name="ps1", bufs=4, space="PSUM"))
            ps2 = ph.enter_context(
                tc.tile_pool(name="ps2", bufs=1, space="PSUM"))

            g2b2T = const.tile([P, KT], F32, name='g2b2T')
            nc.vector.tensor_mul(g2b2T[:], gmlpT[:], b2T[:])

            for g2 in range(4):           # 4 column groups of 1024
                for dot in range(8):
                    m = 8 * g2 + dot
                    p = ps1.tile([P, NQ], F32, tag="m1", name='m1')
                    for k in range(KT):
                        nc.tensor.matmul(
                            p[:],
                            lhsT=_r(w1t[g2][k][:, dot * P:(dot + 1) * P]),
                            rhs=_r(h2t[k][:]),
                            start=(k == 0), stop=(k == KT - 1))
                    nc.scalar.activation(gTt[m][:], p[:], AF.Gelu_apprx_tanh,
                                         bias=b1T[:, m:m + 1])

            for half in range(2):
                pacc = [ps2.tile([P, NQ], F32, tag=f"acc{d}",
                                 name=f"acc{d}") for d in range(4)]
                for mk in range(MT):
                    w2c = w2pool.tile([P, NQ], BF16, tag="w2", name='w2')
                    eng = nc.sync if mk % 2 == 0 else nc.gpsimd
                    eng.dma_start(
                        w2c[:], W2[mk * P:(mk + 1) * P,
                                   half * NQ:(half + 1) * NQ])
                    for d in range(4):
                        nc.tensor.matmul(
                            pacc[d][:],
                            lhsT=_r(w2c[:, d * P:(d + 1) * P]),
                            rhs=_r(gTt[mk][:]),
                            start=(mk == 0), stop=(mk == MT - 1))
                for d in range(4):
                    t = half * 4 + d
                    yt = opool.tile([P, NQ], BF16, tag="yout", name='yout')
                    nc.vector.affine_then_add(
                        yt[:], pacc[d][:], x1t[t][:],
                        scale=gmlpT[:, t:t + 1], bias=g2b2T[:, t:t + 1])
                    nc.scalar.dma_start(yT[t * P:(t + 1) * P, :], yt[:])

        w1_cm.__exit__(None, None, None)
        wo_cm.__exit__(None, None, None)
        op_cm.__exit__(None, None, None)

    nc.compile()
    return nc


_NC = None


def _get_nc():
    global _NC
    if _NC is None:
        _NC = build()
    return _NC


def _prep_inputs(x, c, Wq, bq, Wkv, bkv, Wo, bo, W1, b1, W2, b2, Wada, bada):
    import ml_dtypes
    f = np.float32
    bf = ml_dtypes.bfloat16
    col = lambda v, n: np.ascontiguousarray(
        np.asarray(v, f).reshape(n, P).T)
    shared = {
        "Wq": np.asarray(Wq, f).astype(bf), "Wkv": np.asarray(Wkv, f).astype(bf),
        "Wo": np.asarray(Wo, f).astype(bf), "W1": np.asarray(W1, f).astype(bf),
        "W2": np.asarray(W2, f).astype(bf), "Wada": np.asarray(Wada, f).astype(bf),
        "bada_r": np.asarray(bada, f).reshape(1, -1),
        "bq_c": col(bq, KT), "bk_c": col(np.asarray(bkv, f)[:D], KT),
        "bv_c": col(np.asarray(bkv, f)[D:], KT).astype(bf),
        "bo_r": np.asarray(bo, f).reshape(1, -1),
        "b1_c": col(b1, MT), "b2_c": col(b2, KT),
    }
    in_maps = []
    for core in range(NCORES):
        b, half = core // 2, core % 2
        xb = np.asarray(x[b], f)
        perm = np.concatenate(
            [xb[half * NQ:(half + 1) * NQ],
             xb[(1 - half) * NQ:(2 - half) * NQ]], axis=0)
        m = dict(shared)
        m["xT"] = np.ascontiguousarray(perm.T).astype(bf)
        m["crow"] = np.asarray(c[b:b + 1], f)
        in_maps.append(m)
    return in_maps


def _run(inputs, trace=False):
    nc = _get_nc()
    in_maps = _prep_inputs(**inputs)
    res = run_bass_kernel_spmd(nc, in_maps, core_ids=list(range(NCORES)),
                               trace=trace)
    B = 4
    y = np.empty((B, N, D), np.float32)
    for core in range(NCORES):
        b, half = core // 2, core % 2
        y[b, half * NQ:(half + 1) * NQ, :] = (
            res.results[core]["yT"].astype(np.float32).T)
    return y, res


def kernel(**inputs):
    y, _ = _run(inputs, trace=False)
    return y

